# revision 17
# baseline (speedup 1.0000x reference)
"""Trainium2 Bass kernel for CrossAttention (B=4, QL=KL=2048, D=1024, fp32).

reference:
    query = hidden_states @ Wq                      # [B, QL, D]
    kv    = decoder_hidden_states @ Wkv             # [B, KL, 2D]
    key, value = split(kv, 2, axis=-1)
    scores = einsum('bqd,bkd->bqk', query, key) / sqrt(D)
    w = softmax(scores, axis=-1)
    out = einsum('bqk,bkd->bqd', w, value)          # [B, QL, D]

Sharding: 8 cores = batch(4) x q-half(2).  Each core owns 1024 query rows of
one batch.  The K/V projection for a batch is split between the two cores of
the pair by key rows (each computes 1024 of 2048 keys) and the halves are
exchanged with a pair-wise HBM AllGather, so no projection work is
duplicated.  The program is parity-agnostic: the host feeds each core only
its half of decT, the core computes its local K/V half, and both halves are
read back from the AllGather output (replica-group rank order == global key
order).

All data is fp16 (11-bit mantissa: ~0.05% quantization error, well inside the
2e-2 gate).  fp16 matmuls stream at 1 row/cycle like fp32r but their weight
loads use the fast-weight-load path and half the SBUF/DMA footprint, so
everything stays resident in SBUF.

Attention is computed with TRANSPOSED scores: S^T[k, q] = K @ Q^T directly
via lhsT=KT-slice [d,k], rhs=QT [d, q-512].  exp(S^T) on ACT lands in exactly
the [k, q] stationary layout the AV matmul needs, so there are NO DVE
transposes and NO casts anywhere.  Row sums for softmax come from one extra
1-column matmul against a ones vector that reuses the AV stationary, and
normalization is folded into the ACT copy that drains AV psum.

Phase order: warmup -> A1 (local K proj, then K AllGather) -> A2 (local V
proj, then V AllGather) -> B (Q proj) -> C (scores both q-chunks, then AV).
DMA is critical-first: A1's inputs, then bulk prefetch behind compute; the
exchange bounce DMAs are queued after all input loads so they never stall
the ring.

This walrus build allows only ONE embedded semaphore wait per hardware
instruction; legalize_waits() splits any extra waits onto injected
same-engine NOPs after Tile scheduling.
"""

import sys

if "/opt/trn_rl_repo" not in sys.path:
    sys.path.insert(0, "/opt/trn_rl_repo")

import numpy as np

import bass_rust
import concourse.bass as bass
import concourse.mybir as mybir
import concourse.tile as tile
from concourse.bass_utils import run_bass_kernel_spmd

F32 = mybir.dt.float32
F16 = mybir.dt.float16
EXP = mybir.ActivationFunctionType.Exp
ACOPY = mybir.ActivationFunctionType.Copy

N_CORES = 8
B, QL, KL, D = 4, 2048, 2048, 1024
PAIRS = [[2 * i, 2 * i + 1] for i in range(N_CORES // 2)]


def legalize_waits(nc, max_waits=1):
    """TRN2 instructions embed at most one semaphore wait.  Move excess waits
    emitted by Tile onto same-engine NOPs inserted just before the owning
    instruction (engine FIFO makes this semantically identical)."""
    cnt = 0
    for fn in nc.m.functions:
        for bb in fn.blocks:
            out = []
            changed = False
            for ins in bb.instructions:
                si = ins.sync_info
                if si is not None and si.on_wait and len(si.on_wait) > max_waits:
                    waits = list(si.on_wait)
                    for w in waits[:-max_waits]:
                        cnt += 1
                        nop = bass_rust.InstNoOp(name=f"I-wfix-{cnt}")
                        nop.engine = ins.engine
                        nop.sync_info = mybir.SyncInfo(on_wait=[w], on_update=[])
                        out.append(nop)
                    ins.sync_info = mybir.SyncInfo(
                        on_wait=waits[-max_waits:],
                        on_update=list(si.on_update or []),
                    )
                    changed = True
                out.append(ins)
            if changed:
                bb.instructions = out
    return cnt


def build_attention(nc, QS, KLp, Dp, scale):
    DS = Dp // 128          # contraction subtiles
    NDO = Dp // 128         # output-d 128-chunks
    NKT = KLp // 128        # k 128-chunks
    NKL = NKT // 2          # local k 128-chunks (half of the batch's keys)
    NQT = QS // 128         # q 128-chunks
    NQC = QS // 512         # q 512-chunks
    KLH = KLp // 2          # local key count
    BLK = DS * 128          # free extent of one [128, DS*128] DRAM block

    # block-layout params: [nblk, 128, DS*128], fp16.  decT holds only this
    # core's half of the batch's keys (host slices per core parity).
    hsT = nc.declare_dram_parameter("hsT", [NQT, 128, BLK], F16, isOutput=False)
    decT = nc.declare_dram_parameter("decT", [NKL, 128, BLK], F16, isOutput=False)
    wq = nc.declare_dram_parameter("wq", [NDO, 128, BLK], F16, isOutput=False)
    wkv = nc.declare_dram_parameter("wkv", [2 * NDO, 128, BLK], F16, isOutput=False)
    # fp16 transport for the output; the host upcasts to f32 (0.05% quant
    # error, negligible vs the 2e-2 gate) — halves the output DMA + tail.
    out = nc.declare_dram_parameter("out", [QS, Dp], F16, isOutput=True)

    def load_blocks(dst, src, blk0, nblk):
        """One DMA moving nblk consecutive [128, BLK] DRAM blocks into an
        SBUF tile laid out [128, nblk, DS, 128]."""
        if nblk == 1:
            nc.sync.dma_start(
                dst[:], src[blk0].rearrange("p (s o) -> p s o", o=128)
            )
        else:
            nc.sync.dma_start(
                dst.rearrange("p b s o -> p b (s o)"),
                src[blk0 : blk0 + nblk].rearrange("b p f -> p b f"),
            )

    with tile.TileContext(nc) as tc:
        pools = []

        def enter(cm):
            pools.append(cm)
            return cm.__enter__()

        def close(cm):
            pools.remove(cm)
            cm.__exit__(None, None, None)

        # long-lived pools on the RIGHT stack; transient per-phase pools on
        # the LEFT stack close LIFO at phase boundaries.
        constp_cm = tc.tile_pool(name="const", bufs=1, side="right")
        ktp_cm = tc.tile_pool(name="ktp", bufs=1, side="right")
        vp_cm = tc.tile_pool(name="vp", bufs=1, side="right")
        qtp_cm = tc.tile_pool(name="qtp", bufs=1, side="right")
        dramp_cm = tc.tile_pool(name="dram", bufs=1, space="DRAM")

        wqp_cm = tc.tile_pool(name="wqp", bufs=1)
        htp_cm = tc.tile_pool(name="htp", bufs=1)
        wlop_cm = tc.tile_pool(name="wlo", bufs=1)
        whip_cm = tc.tile_pool(name="whi", bufs=1)
        decp_cm = tc.tile_pool(name="dec", bufs=1)

        psP_cm = tc.tile_pool(name="psP", bufs=4, space="PSUM")

        constp = enter(constp_cm)
        ktp = enter(ktp_cm)
        vp = enter(vp_cm)
        qtp = enter(qtp_cm)
        dramp = enter(dramp_cm)
        wqp = enter(wqp_cm)
        htp = enter(htp_cm)
        wlop = enter(wlop_cm)
        whip = enter(whip_cm)
        decp = enter(decp_cm)
        psP = enter(psP_cm)

        # constants: ones column (for row sums) + warmup tile
        ones = constp.tile([128, 8], F16)
        warm = constp.tile([128, 640], F16)
        nc.gpsimd.memset(ones[:], 1.0)
        nc.gpsimd.memset(warm[:, 0:128], 1.0)
        nc.vector.memset(warm[:, 128:640], 1.0)

        # HAM warmup: keep the PE busy during the initial DMA wave so the
        # clock gate is at 8/8 when phase A1's first real matmul issues.
        warm_ps_cm = tc.tile_pool(name="wps", bufs=1, space="PSUM")
        warm_ps_pool = enter(warm_ps_cm)
        warm_ps = warm_ps_pool.tile([128, 512], F32)
        for _ in range(22):
            nc.tensor.matmul(
                warm_ps[:], warm[:, 0:128], warm[:, 128:640],
                start=True, stop=True, skip_group_check=True,
            )
        close(warm_ps_cm)

        # input SBUF tiles
        wqt = wqp.tile([128, NDO, DS, 128], F16, tag="wqp")
        ht = htp.tile([128, NQT, DS, 128], F16, tag="htp")
        wlo = wlop.tile([128, NDO, DS, 128], F16, tag="wlo")
        whi = whip.tile([128, NDO, DS, 128], F16, tag="whi")
        dect = decp.tile([128, NKL, DS, 128], F16, tag="dec")

        # critical-first loads: A1's inputs, then everything else.  All
        # input loads are queued before the exchange bounce DMAs so the
        # (FIFO) ring never stalls on a compute dependency.
        load_blocks(dect[:, 0:4], decT, 0, 4)
        load_blocks(wlo[:], wkv, 0, NDO)
        load_blocks(dect[:, 4:8], decT, 4, 4)
        load_blocks(whi[:], wkv, NDO, NDO)
        load_blocks(wqt[:], wq, 0, NDO)
        load_blocks(ht[:, 0:4], hsT, 0, 4)
        load_blocks(ht[:, 4:8], hsT, 4, 4)

        # long-lived SBUF tensors
        QT = qtp.tile([128, DS, QS], F16, tag="QT")      # [d, q] rhs for scores
        KT = ktp.tile([128, DS, KLp], F16, tag="KT")     # [d, k] lhsT for scores
        V = vp.tile([128, NKT, Dp], F16, tag="V")        # [k, d] rhs for AV

        # DRAM bounce buffers for the pair exchange.  The exchange is a
        # pairwise AllReduce(add): each core receives own+peer summed, puts
        # it in the peer region, and one DVE subtract of the own half
        # recovers the peer half (fp16 cancellation error ~2^-11, harmless).
        # This is parity-free: K/V live in LOCAL-FIRST key order (own half
        # at k 0:KLH, peer at KLH:), which is valid because softmax and AV
        # are permutation-invariant over keys as long as KT and V use the
        # same ordering.
        kt_in_b = dramp.tile([128, DS, KLH], F16, tag="ktib")
        kt_out_b = dramp.tile([128, DS, KLH], F16, tag="ktob")
        v_in_b = dramp.tile([128, NKL, Dp], F16, tag="vib")
        v_out_b = dramp.tile([128, NKL, Dp], F16, tag="vob")

        # -------- Phase A1: local KT[do, k] = Wkv_lo^T @ decT_local --------
        for kc in range(2):
            for do in range(NDO):
                ps = psP.tile([128, 512], F32, tag="psP")
                for di in range(DS):
                    nc.tensor.matmul(
                        ps[:], wlo[:, do, di, :],
                        dect[:, 4 * kc : 4 * kc + 4, di, :],
                        start=(di == 0), stop=(di == DS - 1),
                    )
                nc.vector.tensor_copy(
                    KT[:, do, 512 * kc : 512 * kc + 512], ps[:]
                )
        # K exchange: local half -> bounce -> AllReduce(add) -> peer region
        nc.sync.dma_start(kt_in_b[:], KT[:, :, 0:KLH])
        nc.gpsimd.collective_compute(
            "AllReduce", mybir.AluOpType.add,
            replica_groups=PAIRS,
            ins=[kt_in_b[:]], outs=[kt_out_b[:]],
        )

        # -------- Phase A2: local V[k, d] = decT_local^T @ Wkv_hi ----------
        for kt in range(NKL):
            ps0 = psP.tile([128, 512], F32, tag="psP")
            ps1 = psP.tile([128, 512], F32, tag="psP")
            for di in range(DS):
                nc.tensor.matmul(
                    ps0[:], dect[:, kt, di, :], whi[:, 0:4, di, :],
                    start=(di == 0), stop=(di == DS - 1),
                )
                nc.tensor.matmul(
                    ps1[:], dect[:, kt, di, :], whi[:, 4:8, di, :],
                    start=(di == 0), stop=(di == DS - 1),
                )
            nc.vector.tensor_copy(V[:, kt, 0:512], ps0[:])
            nc.vector.tensor_copy(V[:, kt, 512:1024], ps1[:])
        # V exchange
        nc.sync.dma_start(v_in_b[:], V[:, 0:NKL, :])
        nc.gpsimd.collective_compute(
            "AllReduce", mybir.AluOpType.add,
            replica_groups=PAIRS,
            ins=[v_in_b[:]], outs=[v_out_b[:]],
        )
        # write-backs (peer region <- summed halves), then subtract own half
        nc.sync.dma_start(KT[:, :, KLH:KLp], kt_out_b[:])
        nc.vector.tensor_tensor(
            KT[:, :, KLH:KLp], KT[:, :, KLH:KLp], KT[:, :, 0:KLH],
            mybir.AluOpType.subtract,
        )
        nc.sync.dma_start(V[:, NKL:NKT, :], v_out_b[:])
        nc.vector.tensor_tensor(
            V[:, NKL:NKT, :], V[:, NKL:NKT, :], V[:, 0:NKL, :],
            mybir.AluOpType.subtract,
        )

        # ---------------- Phase B: QT[do, q] = Wq^T @ hsT ------------------
        for do in range(NDO):
            ps0 = psP.tile([128, 512], F32, tag="psP")
            ps1 = psP.tile([128, 512], F32, tag="psP")
            for di in range(DS):
                nc.tensor.matmul(
                    ps0[:], wqt[:, do, di, :], ht[:, 0:4, di, :],
                    start=(di == 0), stop=(di == DS - 1),
                )
                nc.tensor.matmul(
                    ps1[:], wqt[:, do, di, :], ht[:, 4:8, di, :],
                    start=(di == 0), stop=(di == DS - 1),
                )
            nc.vector.tensor_copy(QT[:, do, 0:512], ps0[:])
            nc.vector.tensor_copy(QT[:, do, 512:1024], ps1[:])
        close(psP_cm)
        close(decp_cm)
        close(whip_cm)
        close(wlop_cm)
        close(htp_cm)
        close(wqp_cm)

        # ---------------- Phase C: attention ------------------------------
        ptp_cm = tc.tile_pool(name="ptp", bufs=NQC, side="right")
        statp_cm = tc.tile_pool(name="stat", bufs=4, side="right")
        ostp_cm = tc.tile_pool(name="ost", bufs=2, side="right")
        ps_sc_cm = tc.tile_pool(name="ps_sc", bufs=3, space="PSUM")
        ps_av_cm = tc.tile_pool(name="ps_av", bufs=4, space="PSUM")
        ls_cm = tc.tile_pool(name="ls", bufs=1, space="PSUM")
        ptp = enter(ptp_cm)
        statp = enter(statp_cm)
        ostp = enter(ostp_cm)
        ps_sc = enter(ps_sc_cm)
        ps_av = enter(ps_av_cm)
        lsp = enter(ls_cm)
        PT = [
            ptp.tile([128, NKT, 512], F16, tag="ptp", name=f"PT{c}")
            for c in range(NQC)
        ]
        ls = lsp.tile([128, 64], F32, tag="ls")

        # scores^T + exp for both q-chunks: S^T[k, q] = K @ Q^T
        for kt in range(NKT):
            pscs = []
            for c in range(NQC):
                ps = ps_sc.tile([128, 512], F32, tag="ps_sc")
                pscs.append(ps)
            for di in range(DS):
                for c in range(NQC):
                    nc.tensor.matmul(
                        pscs[c][:],
                        KT[:, di, 128 * kt : 128 * kt + 128],
                        QT[:, di, 512 * c : 512 * c + 512],
                        start=(di == 0), stop=(di == DS - 1),
                    )
            for c in range(NQC):
                nc.scalar.activation(
                    PT[c][:, kt, :], pscs[c][:], EXP,
                    bias=0.0, scale=float(scale),
                )

        # AV + row-sum + normalize per 128-q subtile
        for c in range(NQC):
            for qs in range(4):
                last = c == NQC - 1 and qs == 3
                av0 = ps_av.tile([128, 512], F32, tag="ps_av")
                av1 = ps_av.tile([128, 512], F32, tag="ps_av")
                col = 8 * (4 * c + qs)
                recip = statp.tile([128, 1], F32, tag="stat")
                ot = ostp.tile([128, Dp], F16, tag="ost")
                qrow = (4 * c + qs) * 128

                def pt_lhsT(kt, c=c, qs=qs):
                    return PT[c][:, kt, 128 * qs : 128 * qs + 128]

                if not last:
                    for kt in range(NKT):
                        lhsT = pt_lhsT(kt)
                        nc.tensor.matmul(
                            av0[:], lhsT, V[:, kt, 0:512],
                            start=(kt == 0), stop=(kt == NKT - 1),
                        )
                        nc.tensor.matmul(
                            av1[:], lhsT, V[:, kt, 512:1024],
                            start=(kt == 0), stop=(kt == NKT - 1),
                        )
                        nc.tensor.matmul(
                            ls[:, col : col + 1], lhsT, ones[:, 0:1],
                            start=(kt == 0), stop=(kt == NKT - 1),
                        )
                    nc.vector.reciprocal(recip[:], ls[:, col : col + 1])
                    nc.scalar.activation(
                        ot[:, 0:512], av0[:], ACOPY, bias=0.0, scale=recip[:],
                    )
                    nc.scalar.activation(
                        ot[:, 512:1024], av1[:], ACOPY, bias=0.0, scale=recip[:],
                    )
                    nc.sync.dma_start(out[qrow : qrow + 128, :], ot[:])
                else:
                    # last subtile: row-sum matmuls first so the reciprocal
                    # computes during AV, and av0 drains + ships while av1 is
                    # still accumulating — shortens the kernel tail.
                    for kt in range(NKT):
                        nc.tensor.matmul(
                            ls[:, col : col + 1], pt_lhsT(kt), ones[:, 0:1],
                            start=(kt == 0), stop=(kt == NKT - 1),
                        )
                    nc.vector.reciprocal(recip[:], ls[:, col : col + 1])
                    for kt in range(NKT):
                        nc.tensor.matmul(
                            av0[:], pt_lhsT(kt), V[:, kt, 0:512],
                            start=(kt == 0), stop=(kt == NKT - 1),
                        )
                    nc.scalar.activation(
                        ot[:, 0:512], av0[:], ACOPY, bias=0.0, scale=recip[:],
                    )
                    nc.sync.dma_start(out[qrow : qrow + 128, 0:512], ot[:, 0:512])
                    for kt in range(NKT):
                        nc.tensor.matmul(
                            av1[:], pt_lhsT(kt), V[:, kt, 512:1024],
                            start=(kt == 0), stop=(kt == NKT - 1),
                        )
                    nc.scalar.activation(
                        ot[:, 512:1024], av1[:], ACOPY, bias=0.0, scale=recip[:],
                    )
                    nc.sync.dma_start(
                        out[qrow : qrow + 128, 512:1024], ot[:, 512:1024]
                    )

        for cm in list(reversed(pools)):
            close(cm)

    legalize_waits(nc)
    return nc


def _pack_dT_blocks(x, DS):
    """[N, Dp] -> [N//128, 128, DS*128] where block b holds
    res[b, p, s*128+o] = x[b*128+o, s*128+p]  (partitions carry d, free
    carries (subtile s, n-within-block)).  fp16 output."""
    N, Dp = x.shape
    r = x.reshape(N // 128, 128, DS, 128).transpose(0, 3, 2, 1)
    return np.ascontiguousarray(r.reshape(N // 128, 128, DS * 128).astype(np.float16))


def prepare_in_maps(hidden_states, decoder_hidden_states, Wq, Wkv):
    hidden_states = np.asarray(hidden_states, dtype=np.float32)
    decoder_hidden_states = np.asarray(decoder_hidden_states, dtype=np.float32)
    Wq = np.asarray(Wq, dtype=np.float32)
    Wkv = np.asarray(Wkv, dtype=np.float32)
    QS = QL // 2
    KLH = KL // 2
    DS = D // 128

    wq_p = _pack_dT_blocks(Wq.T, DS)      # [do][p, s*128+o] = Wq[s*128+p, do*128+o]
    wkv_p = _pack_dT_blocks(Wkv.T, DS)

    in_maps = []
    for c in range(N_CORES):
        b, h = c // 2, c % 2
        hs = hidden_states[b, h * QS : (h + 1) * QS]          # [QS, D]
        dec = decoder_hidden_states[b, h * KLH : (h + 1) * KLH]  # local key half
        in_maps.append(
            {
                "hsT": _pack_dT_blocks(hs, DS),    # [NQT, 128, DS*128]
                "decT": _pack_dT_blocks(dec, DS),  # [NKL, 128, DS*128]
                "wq": wq_p,
                "wkv": wkv_p,
            }
        )
    return in_maps


def kernel(hidden_states, decoder_hidden_states, Wq, Wkv):
    QS = QL // 2
    scale = 1.0 / float(np.sqrt(D))

    nc = bass.Bass(num_devices=N_CORES)
    build_attention(nc, QS, KL, D, scale)
    in_maps = prepare_in_maps(hidden_states, decoder_hidden_states, Wq, Wkv)

    res = run_bass_kernel_spmd(nc, in_maps, list(range(N_CORES)))

    out = np.empty((B, QL, D), dtype=np.float32)
    for c in range(N_CORES):
        b, h = c // 2, c % 2
        out[b, h * QS : (h + 1) * QS] = res.results[c]["out"]
    return out


# revision 22
# speedup vs baseline: 1.2431x; 1.2431x over previous
"""Trainium2 Bass kernel for CrossAttention (B=4, QL=KL=2048, D=1024, fp32).

reference:
    query = hidden_states @ Wq                      # [B, QL, D]
    kv    = decoder_hidden_states @ Wkv             # [B, KL, 2D]
    key, value = split(kv, 2, axis=-1)
    scores = einsum('bqd,bkd->bqk', query, key) / sqrt(D)
    w = softmax(scores, axis=-1)
    out = einsum('bqk,bkd->bqd', w, value)          # [B, QL, D]

Sharding: 8 cores = batch(4) x q-half(2).  Each core owns 1024 query rows of
one batch.  The K/V projection for a batch is split between the two cores of
the pair by key rows (each computes 1024 of 2048 keys) and the halves are
exchanged with a pair-wise HBM AllGather, so no projection work is
duplicated.  The program is parity-agnostic: the host feeds each core only
its half of decT, the core computes its local K/V half, and both halves are
read back from the AllGather output (replica-group rank order == global key
order).

All data is fp16 (11-bit mantissa: ~0.05% quantization error, well inside the
2e-2 gate).  fp16 matmuls stream at 1 row/cycle like fp32r but their weight
loads use the fast-weight-load path and half the SBUF/DMA footprint, so
everything stays resident in SBUF.

Attention is computed with TRANSPOSED scores: S^T[k, q] = K @ Q^T directly
via lhsT=KT-slice [d,k], rhs=QT [d, q-512].  exp(S^T) on ACT lands in exactly
the [k, q] stationary layout the AV matmul needs, so there are NO DVE
transposes and NO casts anywhere.  Row sums for softmax come from one extra
1-column matmul against a ones vector that reuses the AV stationary, and
normalization is folded into the ACT copy that drains AV psum.

Phase order: warmup -> A1 (local K proj, then K AllGather) -> A2 (local V
proj, then V AllGather) -> B (Q proj) -> C (scores both q-chunks, then AV).
DMA is critical-first: A1's inputs, then bulk prefetch behind compute; the
exchange bounce DMAs are queued after all input loads so they never stall
the ring.

This walrus build allows only ONE embedded semaphore wait per hardware
instruction; legalize_waits() splits any extra waits onto injected
same-engine NOPs after Tile scheduling.
"""

import sys

if "/opt/trn_rl_repo" not in sys.path:
    sys.path.insert(0, "/opt/trn_rl_repo")

import numpy as np

import bass_rust
import concourse.bass as bass
import concourse.mybir as mybir
import concourse.tile as tile
from concourse.bass_utils import run_bass_kernel_spmd

F32 = mybir.dt.float32
F16 = mybir.dt.float16
EXP = mybir.ActivationFunctionType.Exp
ACOPY = mybir.ActivationFunctionType.Copy

N_CORES = 8
B, QL, KL, D = 4, 2048, 2048, 1024
PAIRS = [[2 * i, 2 * i + 1] for i in range(N_CORES // 2)]


def legalize_waits(nc, max_waits=1):
    """TRN2 instructions embed at most one semaphore wait.  Move excess waits
    emitted by Tile onto same-engine NOPs inserted just before the owning
    instruction (engine FIFO makes this semantically identical)."""
    cnt = 0
    for fn in nc.m.functions:
        for bb in fn.blocks:
            out = []
            changed = False
            for ins in bb.instructions:
                si = ins.sync_info
                if si is not None and si.on_wait and len(si.on_wait) > max_waits:
                    waits = list(si.on_wait)
                    for w in waits[:-max_waits]:
                        cnt += 1
                        nop = bass_rust.InstNoOp(name=f"I-wfix-{cnt}")
                        nop.engine = ins.engine
                        nop.sync_info = mybir.SyncInfo(on_wait=[w], on_update=[])
                        out.append(nop)
                    ins.sync_info = mybir.SyncInfo(
                        on_wait=waits[-max_waits:],
                        on_update=list(si.on_update or []),
                    )
                    changed = True
                out.append(ins)
            if changed:
                bb.instructions = out
    return cnt


def build_attention(nc, QS, KLp, Dp, scale):
    DS = Dp // 128          # contraction subtiles
    NDO = Dp // 128         # output-d 128-chunks
    NKT = KLp // 128        # k 128-chunks
    NKL = NKT // 2          # local k 128-chunks (half of the batch's keys)
    NQT = QS // 128         # q 128-chunks
    NQC = QS // 512         # q 512-chunks
    KLH = KLp // 2          # local key count
    BLK = DS * 128          # free extent of one [128, DS*128] DRAM block

    # block-layout params: [nblk, 128, DS*128], fp16.  decT holds only this
    # core's half of the batch's keys (host slices per core parity).
    hsT = nc.declare_dram_parameter("hsT", [NQT, 128, BLK], F16, isOutput=False)
    decT = nc.declare_dram_parameter("decT", [NKL, 128, BLK], F16, isOutput=False)
    wq = nc.declare_dram_parameter("wq", [NDO, 128, BLK], F16, isOutput=False)
    wkv = nc.declare_dram_parameter("wkv", [2 * NDO, 128, BLK], F16, isOutput=False)
    # fp16 transport for the output; the host upcasts to f32 (0.05% quant
    # error, negligible vs the 2e-2 gate) — halves the output DMA + tail.
    out = nc.declare_dram_parameter("out", [QS, Dp], F16, isOutput=True)

    def load_blocks(dst, src, blk0, nblk):
        """One DMA moving nblk consecutive [128, BLK] DRAM blocks into an
        SBUF tile laid out [128, nblk, DS, 128]."""
        if nblk == 1:
            nc.sync.dma_start(
                dst[:], src[blk0].rearrange("p (s o) -> p s o", o=128)
            )
        else:
            nc.sync.dma_start(
                dst.rearrange("p b s o -> p b (s o)"),
                src[blk0 : blk0 + nblk].rearrange("b p f -> p b f"),
            )

    with tile.TileContext(nc) as tc:
        pools = []

        def enter(cm):
            pools.append(cm)
            return cm.__enter__()

        def close(cm):
            pools.remove(cm)
            cm.__exit__(None, None, None)

        # long-lived pools on the RIGHT stack; transient per-phase pools on
        # the LEFT stack close LIFO at phase boundaries.
        constp_cm = tc.tile_pool(name="const", bufs=1, side="right")
        ktp_cm = tc.tile_pool(name="ktp", bufs=1, side="right")
        vp_cm = tc.tile_pool(name="vp", bufs=1, side="right")
        qtp_cm = tc.tile_pool(name="qtp", bufs=1, side="right")
        dramp_cm = tc.tile_pool(name="dram", bufs=1, space="DRAM")

        wqp_cm = tc.tile_pool(name="wqp", bufs=1)
        htp_cm = tc.tile_pool(name="htp", bufs=1)
        wlop_cm = tc.tile_pool(name="wlo", bufs=1)
        whip_cm = tc.tile_pool(name="whi", bufs=1)
        decp_cm = tc.tile_pool(name="dec", bufs=1)

        psP_cm = tc.tile_pool(name="psP", bufs=4, space="PSUM")

        constp = enter(constp_cm)
        ktp = enter(ktp_cm)
        vp = enter(vp_cm)
        qtp = enter(qtp_cm)
        dramp = enter(dramp_cm)
        wqp = enter(wqp_cm)
        htp = enter(htp_cm)
        wlop = enter(wlop_cm)
        whip = enter(whip_cm)
        decp = enter(decp_cm)
        psP = enter(psP_cm)

        # constants: ones column (for row sums) + warmup tile
        ones = constp.tile([128, 8], F16)
        warm = constp.tile([128, 640], F16)
        nc.gpsimd.memset(ones[:], 1.0)
        nc.gpsimd.memset(warm[:, 0:128], 1.0)
        nc.vector.memset(warm[:, 128:640], 1.0)

        # HAM warmup: keep the PE busy during the initial DMA wave so the
        # clock gate is at 8/8 when phase A1's first real matmul issues.
        warm_ps_cm = tc.tile_pool(name="wps", bufs=1, space="PSUM")
        warm_ps_pool = enter(warm_ps_cm)
        warm_ps = warm_ps_pool.tile([128, 512], F32)
        for _ in range(34):
            nc.tensor.matmul(
                warm_ps[:], warm[:, 0:128], warm[:, 128:640],
                start=True, stop=True, skip_group_check=True,
            )
        close(warm_ps_cm)

        # input SBUF tiles
        wqt = wqp.tile([128, NDO, DS, 128], F16, tag="wqp")
        ht = htp.tile([128, NQT, DS, 128], F16, tag="htp")
        wlo = wlop.tile([128, NDO, DS, 128], F16, tag="wlo")
        whi = whip.tile([128, NDO, DS, 128], F16, tag="whi")
        dect = decp.tile([128, NKL, DS, 128], F16, tag="dec")

        # critical-first loads: A1's inputs, then everything else.  All
        # input loads are queued before the exchange bounce DMAs so the
        # (FIFO) ring never stalls on a compute dependency.
        load_blocks(dect[:, 0:4], decT, 0, 4)
        load_blocks(wlo[:], wkv, 0, NDO)
        load_blocks(dect[:, 4:8], decT, 4, 4)
        load_blocks(whi[:], wkv, NDO, NDO)
        load_blocks(wqt[:], wq, 0, NDO)
        load_blocks(ht[:, 0:4], hsT, 0, 4)
        load_blocks(ht[:, 4:8], hsT, 4, 4)

        # long-lived SBUF tensors.  KT is [d-part, half, dsub, k-local]: the
        # own/peer halves are CONTIGUOUS so the exchange write-back and
        # subtract have tight dependency intervals that don't overlap the
        # local half scores are already reading.
        QT = qtp.tile([128, DS, QS], F16, tag="QT")      # [d, q] rhs for scores
        KT = ktp.tile([128, 2, DS, KLH], F16, tag="KT")  # [d, k] lhsT for scores
        V = vp.tile([128, NKT, Dp], F16, tag="V")        # [k, d] rhs for AV

        # DRAM bounce buffers for the pair exchange.  The exchange is a
        # pairwise AllReduce(add): each core receives own+peer summed, puts
        # it in the peer region, and one DVE subtract of the own half
        # recovers the peer half (fp16 cancellation error ~2^-11, harmless).
        # This is parity-free: K/V live in LOCAL-FIRST key order (own half
        # at k 0:KLH, peer at KLH:), which is valid because softmax and AV
        # are permutation-invariant over keys as long as KT and V use the
        # same ordering.
        kt_in_b = dramp.tile([128, DS, KLH], F16, tag="ktib")
        kt_out_b = dramp.tile([128, DS, KLH], F16, tag="ktob")
        v_in_b = dramp.tile([128, NKL, Dp], F16, tag="vib")
        v_out_b = dramp.tile([128, NKL, Dp], F16, tag="vob")

        # -------- Phase A1: local KT[do, k] = Wkv_lo^T @ decT_local --------
        for kc in range(2):
            for do in range(NDO):
                ps = psP.tile([128, 512], F32, tag="psP")
                for di in range(DS):
                    nc.tensor.matmul(
                        ps[:], wlo[:, do, di, :],
                        dect[:, 4 * kc : 4 * kc + 4, di, :],
                        start=(di == 0), stop=(di == DS - 1),
                    )
                nc.vector.tensor_copy(
                    KT[:, 0, do, 512 * kc : 512 * kc + 512], ps[:]
                )
        # K exchange: local half -> bounce -> AllReduce(add) -> peer region
        nc.sync.dma_start(kt_in_b[:], KT[:, 0])
        nc.gpsimd.collective_compute(
            "AllReduce", mybir.AluOpType.add,
            replica_groups=PAIRS,
            ins=[kt_in_b[:]], outs=[kt_out_b[:]],
        )

        # -------- Phase A2: local V[k, d] = decT_local^T @ Wkv_hi ----------
        for kt in range(NKL):
            ps0 = psP.tile([128, 512], F32, tag="psP")
            ps1 = psP.tile([128, 512], F32, tag="psP")
            for di in range(DS):
                nc.tensor.matmul(
                    ps0[:], dect[:, kt, di, :], whi[:, 0:4, di, :],
                    start=(di == 0), stop=(di == DS - 1),
                )
                nc.tensor.matmul(
                    ps1[:], dect[:, kt, di, :], whi[:, 4:8, di, :],
                    start=(di == 0), stop=(di == DS - 1),
                )
            nc.vector.tensor_copy(V[:, kt, 0:512], ps0[:])
            nc.vector.tensor_copy(V[:, kt, 512:1024], ps1[:])
        # V exchange
        nc.sync.dma_start(v_in_b[:], V[:, 0:NKL, :])
        nc.gpsimd.collective_compute(
            "AllReduce", mybir.AluOpType.add,
            replica_groups=PAIRS,
            ins=[v_in_b[:]], outs=[v_out_b[:]],
        )
        # write-backs (peer region <- summed halves), then subtract own half
        # (on GpSimd, which is otherwise idle, so the waits don't block the
        # DVE psum-drain queue)
        nc.sync.dma_start(KT[:, 1], kt_out_b[:])
        nc.gpsimd.tensor_tensor(
            KT[:, 1], KT[:, 1], KT[:, 0],
            mybir.AluOpType.subtract,
        )
        nc.sync.dma_start(V[:, NKL:NKT, :], v_out_b[:])
        nc.gpsimd.tensor_tensor(
            V[:, NKL:NKT, :], V[:, NKL:NKT, :], V[:, 0:NKL, :],
            mybir.AluOpType.subtract,
        )

        # ---------------- Phase B: QT[do, q] = Wq^T @ hsT ------------------
        for do in range(NDO):
            ps0 = psP.tile([128, 512], F32, tag="psP")
            ps1 = psP.tile([128, 512], F32, tag="psP")
            for di in range(DS):
                nc.tensor.matmul(
                    ps0[:], wqt[:, do, di, :], ht[:, 0:4, di, :],
                    start=(di == 0), stop=(di == DS - 1),
                )
                nc.tensor.matmul(
                    ps1[:], wqt[:, do, di, :], ht[:, 4:8, di, :],
                    start=(di == 0), stop=(di == DS - 1),
                )
            nc.vector.tensor_copy(QT[:, do, 0:512], ps0[:])
            nc.vector.tensor_copy(QT[:, do, 512:1024], ps1[:])
        close(psP_cm)
        close(decp_cm)
        close(whip_cm)
        close(wlop_cm)
        close(htp_cm)
        close(wqp_cm)

        # ---------------- Phase C: attention ------------------------------
        ptp_cm = tc.tile_pool(name="ptp", bufs=NQC, side="right")
        statp_cm = tc.tile_pool(name="stat", bufs=4, side="right")
        ostp_cm = tc.tile_pool(name="ost", bufs=2, side="right")
        ps_sc_cm = tc.tile_pool(name="ps_sc", bufs=3, space="PSUM")
        ps_av_cm = tc.tile_pool(name="ps_av", bufs=4, space="PSUM")
        ls_cm = tc.tile_pool(name="ls", bufs=1, space="PSUM")
        ptp = enter(ptp_cm)
        statp = enter(statp_cm)
        ostp = enter(ostp_cm)
        ps_sc = enter(ps_sc_cm)
        ps_av = enter(ps_av_cm)
        lsp = enter(ls_cm)
        PT = [
            ptp.tile([128, NKT, 512], F16, tag="ptp", name=f"PT{c}")
            for c in range(NQC)
        ]
        ls = lsp.tile([128, 64], F32, tag="ls")

        # scores^T + exp for both q-chunks: S^T[k, q] = K @ Q^T
        for kt in range(NKT):
            pscs = []
            for c in range(NQC):
                ps = ps_sc.tile([128, 512], F32, tag="ps_sc")
                pscs.append(ps)
            for di in range(DS):
                for c in range(NQC):
                    nc.tensor.matmul(
                        pscs[c][:],
                        KT[:, kt // NKL, di,
                           128 * (kt % NKL) : 128 * (kt % NKL) + 128],
                        QT[:, di, 512 * c : 512 * c + 512],
                        start=(di == 0), stop=(di == DS - 1),
                    )
            for c in range(NQC):
                nc.scalar.activation(
                    PT[c][:, kt, :], pscs[c][:], EXP,
                    bias=0.0, scale=float(scale),
                )

        # AV + row-sum + normalize per 128-q subtile
        for c in range(NQC):
            for qs in range(4):
                last = c == NQC - 1 and qs == 3
                av0 = ps_av.tile([128, 512], F32, tag="ps_av")
                av1 = ps_av.tile([128, 512], F32, tag="ps_av")
                col = 8 * (4 * c + qs)
                recip = statp.tile([128, 1], F32, tag="stat")
                ot = ostp.tile([128, Dp], F16, tag="ost")
                qrow = (4 * c + qs) * 128

                def pt_lhsT(kt, c=c, qs=qs):
                    return PT[c][:, kt, 128 * qs : 128 * qs + 128]

                if not last:
                    for kt in range(NKT):
                        lhsT = pt_lhsT(kt)
                        nc.tensor.matmul(
                            av0[:], lhsT, V[:, kt, 0:512],
                            start=(kt == 0), stop=(kt == NKT - 1),
                        )
                        nc.tensor.matmul(
                            av1[:], lhsT, V[:, kt, 512:1024],
                            start=(kt == 0), stop=(kt == NKT - 1),
                        )
                        nc.tensor.matmul(
                            ls[:, col : col + 1], lhsT, ones[:, 0:1],
                            start=(kt == 0), stop=(kt == NKT - 1),
                        )
                    nc.vector.reciprocal(recip[:], ls[:, col : col + 1])
                    nc.scalar.activation(
                        ot[:, 0:512], av0[:], ACOPY, bias=0.0, scale=recip[:],
                    )
                    nc.scalar.activation(
                        ot[:, 512:1024], av1[:], ACOPY, bias=0.0, scale=recip[:],
                    )
                    nc.sync.dma_start(out[qrow : qrow + 128, :], ot[:])
                else:
                    # last subtile: row-sum matmuls first so the reciprocal
                    # computes during AV, and av0 drains + ships while av1 is
                    # still accumulating — shortens the kernel tail.
                    for kt in range(NKT):
                        nc.tensor.matmul(
                            ls[:, col : col + 1], pt_lhsT(kt), ones[:, 0:1],
                            start=(kt == 0), stop=(kt == NKT - 1),
                        )
                    nc.vector.reciprocal(recip[:], ls[:, col : col + 1])
                    for kt in range(NKT):
                        nc.tensor.matmul(
                            av0[:], pt_lhsT(kt), V[:, kt, 0:512],
                            start=(kt == 0), stop=(kt == NKT - 1),
                        )
                    nc.scalar.activation(
                        ot[:, 0:512], av0[:], ACOPY, bias=0.0, scale=recip[:],
                    )
                    nc.sync.dma_start(out[qrow : qrow + 128, 0:512], ot[:, 0:512])
                    for kt in range(NKT):
                        nc.tensor.matmul(
                            av1[:], pt_lhsT(kt), V[:, kt, 512:1024],
                            start=(kt == 0), stop=(kt == NKT - 1),
                        )
                    nc.scalar.activation(
                        ot[:, 512:1024], av1[:], ACOPY, bias=0.0, scale=recip[:],
                    )
                    nc.sync.dma_start(
                        out[qrow : qrow + 128, 512:1024], ot[:, 512:1024]
                    )

        for cm in list(reversed(pools)):
            close(cm)

    legalize_waits(nc)
    return nc


def _pack_dT_blocks(x, DS):
    """[N, Dp] -> [N//128, 128, DS*128] where block b holds
    res[b, p, s*128+o] = x[b*128+o, s*128+p]  (partitions carry d, free
    carries (subtile s, n-within-block)).  fp16 output."""
    N, Dp = x.shape
    r = x.reshape(N // 128, 128, DS, 128).transpose(0, 3, 2, 1)
    return np.ascontiguousarray(r.reshape(N // 128, 128, DS * 128).astype(np.float16))


def prepare_in_maps(hidden_states, decoder_hidden_states, Wq, Wkv):
    hidden_states = np.asarray(hidden_states, dtype=np.float32)
    decoder_hidden_states = np.asarray(decoder_hidden_states, dtype=np.float32)
    Wq = np.asarray(Wq, dtype=np.float32)
    Wkv = np.asarray(Wkv, dtype=np.float32)
    QS = QL // 2
    KLH = KL // 2
    DS = D // 128

    wq_p = _pack_dT_blocks(Wq.T, DS)      # [do][p, s*128+o] = Wq[s*128+p, do*128+o]
    wkv_p = _pack_dT_blocks(Wkv.T, DS)

    in_maps = []
    for c in range(N_CORES):
        b, h = c // 2, c % 2
        hs = hidden_states[b, h * QS : (h + 1) * QS]          # [QS, D]
        dec = decoder_hidden_states[b, h * KLH : (h + 1) * KLH]  # local key half
        in_maps.append(
            {
                "hsT": _pack_dT_blocks(hs, DS),    # [NQT, 128, DS*128]
                "decT": _pack_dT_blocks(dec, DS),  # [NKL, 128, DS*128]
                "wq": wq_p,
                "wkv": wkv_p,
            }
        )
    return in_maps


def kernel(hidden_states, decoder_hidden_states, Wq, Wkv):
    QS = QL // 2
    scale = 1.0 / float(np.sqrt(D))

    nc = bass.Bass(num_devices=N_CORES)
    build_attention(nc, QS, KL, D, scale)
    in_maps = prepare_in_maps(hidden_states, decoder_hidden_states, Wq, Wkv)

    res = run_bass_kernel_spmd(nc, in_maps, list(range(N_CORES)))

    out = np.empty((B, QL, D), dtype=np.float32)
    for c in range(N_CORES):
        b, h = c // 2, c % 2
        out[b, h * QS : (h + 1) * QS] = res.results[c]["out"]
    return out
